# revision 25
# baseline (speedup 1.0000x reference)
"""CrossModalAttention Trainium2 kernel.

Math (per batch b):
    Q = query @ Wq.T ; K = key @ Wk.T ; V = value @ Wv.T        (per-head dk=64)
    scores = (Q K^T) / sqrt(dk) * cmw[h]   (+ mask)
    attn   = softmax(scores, axis=k)
    ctx    = attn @ V
    out    = ctx @ Wo.T + bo
    y      = LayerNorm(out + query) * gamma + beta
Returns (y, attn).

Sharding: 8 cores = 4 batches x 2 query-halves (512 q rows each). No
collectives: every core computes its q-rows end-to-end (K/V projections are
duplicated across the 2 cores that share a batch).

On-device layout is "transposed" ([feature, token]) so every matmul contracts
over the partition dim with no on-chip transposes:
  - QT[f,q], KT[f,k]  from host-transposed activations and W.T
  - scoresT[k,q] = K_h @ Q_h^T   (lhsT = KT chunk, rhs = QT chunk); the two
    heads of a 128-partition pair use disjoint PE row groups and interleave
  - expT = exp(scoresT * scale)        (ACT, fp16)
  - [ctxT; sums] = [V_h | 1]^T @ expT  (ones column fuses the softmax row-sums)
  - recip broadcast over partitions via a rank-1 matmul (ones outer product)
  - attnT = expT * recip  -> DRAM fp16 (host transposes to [q,k] on gather)
  - ctxT normalized, out = ctx @ Wo.T via lhsT = ctxT chunks
  - residual + LN in natural [q, f] layout (bo pre-folded into the residual
    input on the host).
"""

import math

import numpy as np

import concourse.bass as bass
import concourse.mybir as mybir
import concourse.tile as tile
from concourse import bacc
from concourse.bass_utils import run_bass_kernel_spmd

F32 = mybir.dt.float32
F32R = mybir.dt.float32r
F16 = mybir.dt.float16
AF = mybir.ActivationFunctionType

D = 1024          # d_model
H = 16            # heads
DK = 64           # head dim
B = 4
SQ = 1024
SK = 1024
NCORES = 8
QH = SQ // 2      # q rows per core
LN_EPS = 1e-5

_prog_cache = {}


def _build(scales, use_mask, phases="PAO", repeat=1):
    nc = bacc.Bacc("TRN2", target_bir_lowering=False, debug=False,
                   num_devices=NCORES)

    qT = nc.dram_tensor("qT", [D, QH], F32R, kind="ExternalInput").ap()
    kT = nc.dram_tensor("kT", [D, SK], F32R, kind="ExternalInput").ap()
    vT = nc.dram_tensor("vT", [D, SK], F32R, kind="ExternalInput").ap()
    qnat = nc.dram_tensor("qnat", [QH, D], F32, kind="ExternalInput").ap()
    wqT = nc.dram_tensor("wqT", [D, D], F32R, kind="ExternalInput").ap()
    wkT = nc.dram_tensor("wkT", [D, D], F32R, kind="ExternalInput").ap()
    wvT = nc.dram_tensor("wvT", [D, D], F32R, kind="ExternalInput").ap()
    woT = nc.dram_tensor("woT", [D, D], F32R, kind="ExternalInput").ap()
    gamma = nc.dram_tensor("gamma", [1, D], F32, kind="ExternalInput").ap()
    beta = nc.dram_tensor("beta", [1, D], F32, kind="ExternalInput").ap()
    maskT = None
    if use_mask:
        maskT = nc.dram_tensor("maskT", [SK, QH], F16,
                               kind="ExternalInput").ap()

    attnT = nc.dram_tensor("attnT", [H, SK, QH], F16,
                           kind="ExternalOutput").ap()
    yout = nc.dram_tensor("yout", [QH, D], F32, kind="ExternalOutput").ap()

    def chunked(ap, inner):
        # [ (c p), n ] -> [ p, c, n ] with p=128 partitions
        return ap.rearrange("(c p) n -> p c n", p=128)

    with tile.TileContext(nc) as tc:
      for _rep in range(repeat):
        with tc.tile_pool(name="persist", bufs=1) as pers:
            QT_l = [pers.tile([128, QH], F32R, tag=f"QT{i}", name=f"QT{i}") for i in range(8)]
            KT_l = [pers.tile([128, SK], F32R, tag=f"KT{i}", name=f"KT{i}") for i in range(8)]
            Vo_l = [pers.tile([128, H * 65], F16, tag=f"Vo{i}",
                              name=f"Vo{i}") for i in range(8)]
            ones_sb = pers.tile([1, 128], F32, tag="ones")
            nc.vector.memset(ones_sb, 1.0)
            # ones columns of the [V_h | 1] blocks; V copies overwrite the rest
            for i in range(8):
                nc.vector.memset(Vo_l[i], 1.0)

            # ---------------- Phase P: projections ----------------
            with tc.tile_pool(name="acts", bufs=1) as actp, \
                 tc.tile_pool(name="wts", bufs=1) as wp, \
                 tc.tile_pool(name="vstream", bufs=3) as vsp, \
                 tc.tile_pool(name="pp", bufs=3, space="PSUM") as pp:

                # per-d-chunk DMAs so d=0 matmuls start before the whole
                # tensor has landed
                qT_sb = actp.tile([128, 8, QH], F32R, tag="qT")
                for d in range(8):
                    nc.sync.dma_start(qT_sb[:, d, :], chunked(qT, QH)[:, d, :])
                kT_sb = actp.tile([128, 8, SK], F32R, tag="kT")
                for d in range(8):
                    nc.sync.dma_start(kT_sb[:, d, :], chunked(kT, SK)[:, d, :])

                # QT[f,q] = Wq @ query^T
                wq_sb = wp.tile([128, 8, D], F32R, tag="wA")
                for d in range(8):
                    nc.gpsimd.dma_start(wq_sb[:, d, :], chunked(wqT, D)[:, d, :])
                for fb in range(8):
                    ps = pp.tile([128, SK], F32, tag="pj")
                    for d in range(8):
                        nc.tensor.matmul(
                            ps[:, :QH],
                            lhsT=wq_sb[:, d, fb * 128:(fb + 1) * 128],
                            rhs=qT_sb[:, d, :],
                            start=(d == 0), stop=(d == 7))
                    nc.vector.tensor_copy(QT_l[fb][:], ps[:, :QH])

                # KT[f,k] = Wk @ key^T
                wk_sb = wp.tile([128, 8, D], F32R, tag="wB")
                for d in range(8):
                    nc.gpsimd.dma_start(wk_sb[:, d, :], chunked(wkT, D)[:, d, :])
                for fb in range(8):
                    ps = pp.tile([128, SK], F32, tag="pj")
                    for hf in range(2):
                        for d in range(8):
                            nc.tensor.matmul(
                                ps[:, hf * 512:(hf + 1) * 512],
                                lhsT=wk_sb[:, d, fb * 128:(fb + 1) * 128],
                                rhs=kT_sb[:, d, hf * 512:(hf + 1) * 512],
                                start=(d == 0), stop=(d == 7))
                    nc.vector.tensor_copy(KT_l[fb][:], ps)

                # V[k,f] = value @ Wv.T, written into 65-col [V_h | 1] slots
                wv_sb = wp.tile([128, 8, D], F32R, tag="wA")
                for d in range(8):
                    nc.gpsimd.dma_start(wv_sb[:, d, :], chunked(wvT, D)[:, d, :])
                for kb in range(8):
                    ps = pp.tile([128, SK], F32, tag="pj")
                    vchunk = vsp.tile([128, 8, 128], F32R, tag="vs")
                    nc.sync.dma_start(
                        vchunk, chunked(vT, SK)[:, :, kb * 128:(kb + 1) * 128])
                    for hf in range(2):
                        for d in range(8):
                            nc.tensor.matmul(
                                ps[:, hf * 512:(hf + 1) * 512],
                                lhsT=vchunk[:, d, :],
                                rhs=wv_sb[:, d, hf * 512:(hf + 1) * 512],
                                start=(d == 0), stop=(d == 7))
                    src = ps.rearrange("p (h c) -> p h c", c=DK)
                    dst = Vo_l[kb][:].rearrange("p (h c) -> p h c", c=65)
                    nc.vector.tensor_copy(dst[:, :, 0:DK], src)

            # ---------------- Phases A + O ----------------
            if "A" in phases:
                _phases_ao(nc, tc, QT_l, KT_l, Vo_l, ones_sb,
                           scales, use_mask, maskT, qnat, woT, gamma,
                           beta, attnT, yout, phases, chunked)

    nc.compile()
    return nc


def _head_tail(nc, h, expP, hp, Vo_l, ctxT_sb, attnT, an, pb):
    """ctx+sums matmul, softmax normalization, attnT store for one head."""
    h2 = h // 2
    # [ctxT ; sums] = [V_h | 1]^T @ expT
    ps_ct = pb.tile([128, QH], F32, tag="ct")
    for kb in range(8):
        nc.tensor.matmul(
            ps_ct[0:65, :],
            lhsT=Vo_l[kb][:, h * 65:(h + 1) * 65],
            rhs=expP[:, kb, hp, :],
            start=(kb == 0), stop=(kb == 7))

    # sums live at psum partition 64; recip_approx_fast breaks at
    # partition base != 0 on HW, so hop the row down to partition 0
    # (ACT copy -> tiny DMA) first.
    sums_hi = an.tile([65, QH], F32, tag="sh")
    nc.scalar.copy(sums_hi[64:65, :], ps_ct[64:65, :])
    sums_lo = an.tile([1, QH], F32, tag="sl")
    nc.sync.dma_start(sums_lo[:], sums_hi[64:65, :])
    rec = an.tile([1, QH], F32, tag="rec")
    nc.vector.reciprocal_approx_fast(rec[:], sums_lo[:])
    rec16 = an.tile([1, QH], F16, tag="rec16")
    nc.vector.tensor_copy(rec16[:], rec[:])
    b32 = an.tile([64, QH], F32, tag="b32")
    nc.gpsimd.partition_broadcast(b32[:], rec[:])
    b16 = an.tile([128, QH], F16, tag="b16")
    nc.gpsimd.partition_broadcast(b16[:], rec16[:])

    # normalized ctxT into the head-pair partition slot
    if hp == 0:
        nc.vector.tensor_mul(ctxT_sb[0:64, h2, :], ps_ct[0:64, :],
                             b32[0:64, :])
    else:
        stg = an.tile([64, QH], F32R, tag="stg")
        nc.vector.tensor_mul(stg, ps_ct[0:64, :], b32[0:64, :])
        nc.sync.dma_start(ctxT_sb[64:128, h2, :], stg)

    # attnT out (one batched DMA per head)
    at = an.tile([128, 8, QH], F16, tag="at")
    for kb in range(8):
        nc.vector.tensor_mul(at[:, kb, :], expP[:, kb, hp, :], b16)
    nc.sync.dma_start(attnT[h].rearrange("(c p) q -> p c q", p=128), at)


def _phases_ao(nc, tc, QT_l, KT_l, Vo_l, ones_sb, scales, use_mask,
               maskT, qnat, woT, gamma, beta, attnT, yout, phases, chunked):
    with tc.tile_pool(name="late", bufs=1) as late:
        ctxT_sb = late.tile([128, 8, QH], F32R, tag="ctxT")

        with tc.tile_pool(name="ah", bufs=2) as ah, \
             tc.tile_pool(name="an", bufs=3) as an, \
             tc.tile_pool(name="pa", bufs=3, space="PSUM") as pa, \
             tc.tile_pool(name="pb", bufs=2, space="PSUM") as pb, \
             (tc.tile_pool(name="mp", bufs=1) if use_mask else
              tc.tile_pool(name="mp0", bufs=1)) as mp:

            mask_sb = None
            if use_mask:
                mask_sb = mp.tile([128, 8, QH], F16, tag="mk")
                nc.sync.dma_start(mask_sb, chunked(maskT, QH))

            for h2 in range(8):
                # the two heads of the pair use PE row groups 0-63 / 64-127,
                # so their interleaved scoresT matmuls run concurrently; both
                # land in one 2-bank psum tile so one ACT instruction
                # exponentiates the pair (scales are equal for all heads in
                # the fast path enforced by _prep_inputs).
                expP = ah.tile([128, 8, 2, QH], F16, tag="expP")
                for kb in range(8):
                    ps_s = pa.tile([128, 2, QH], F32, tag="s")
                    for hp in (0, 1):
                        base = hp * 64
                        nc.tensor.matmul(
                            ps_s[:, hp, :],
                            lhsT=KT_l[h2][base:base + 64,
                                          kb * 128:(kb + 1) * 128],
                            rhs=QT_l[h2][base:base + 64, :],
                            start=True, stop=True)
                    if scales[2 * h2] == scales[2 * h2 + 1]:
                        nc.scalar.activation(expP[:, kb, :, :], ps_s,
                                             AF.Exp, scale=scales[2 * h2])
                    else:
                        for hp in (0, 1):
                            nc.scalar.activation(
                                expP[:, kb, hp, :], ps_s[:, hp, :],
                                AF.Exp, scale=scales[2 * h2 + hp])
                    if use_mask:
                        for hp in (0, 1):
                            nc.vector.tensor_mul(expP[:, kb, hp, :],
                                                 expP[:, kb, hp, :],
                                                 mask_sb[:, kb, :])

                for hp in (0, 1):
                    _head_tail(nc, 2 * h2 + hp, expP, hp, Vo_l,
                               ctxT_sb, attnT, an, pb)

        # ---------------- Phase O ----------------
        if "O" not in phases:
            return
        with tc.tile_pool(name="oc", bufs=1) as oc, \
             tc.tile_pool(name="ot", bufs=2) as ot, \
             tc.tile_pool(name="po", bufs=2, space="PSUM") as po:

            wo_sb = oc.tile([128, 8, D], F32R, tag="wo")
            for d in range(8):
                nc.gpsimd.dma_start(wo_sb[:, d, :], chunked(woT, D)[:, d, :])
            qn_sb = oc.tile([128, 4, D], F32, tag="qn")
            for c in range(4):
                nc.sync.dma_start(qn_sb[:, c, :], chunked(qnat, D)[:, c, :])
            ga_sb = oc.tile([128, D], F32, tag="ga")
            nc.sync.dma_start(ga_sb, gamma.to_broadcast([128, D]))
            be_sb = oc.tile([128, D], F32, tag="be")
            nc.sync.dma_start(be_sb, beta.to_broadcast([128, D]))
            eps_sb = oc.tile([128, 1], F32, tag="eps")
            nc.vector.memset(eps_sb, LN_EPS)

            for qb in range(4):
                ps_o = po.tile([128, D], F32, tag="out")
                for hf in range(2):
                    for d in range(8):
                        nc.tensor.matmul(
                            ps_o[:, hf * 512:(hf + 1) * 512],
                            lhsT=ctxT_sb[:, d, qb * 128:(qb + 1) * 128],
                            rhs=wo_sb[:, d, hf * 512:(hf + 1) * 512],
                            start=(d == 0), stop=(d == 7))
                x = ot.tile([128, D], F32, tag="x")
                # qn_sb already holds query-rows + bo (host-folded)
                nc.vector.tensor_add(x, ps_o, qn_sb[:, qb, :])

                stats = ot.tile([128, 2, 6], F32, tag="st")
                for sg in range(2):
                    nc.vector.bn_stats(stats[:, sg, :],
                                       x[:, sg * 512:(sg + 1) * 512])
                mv = ot.tile([128, 2], F32, tag="mv")
                nc.vector.bn_aggr(mv, stats)
                sd = ot.tile([128, 1], F32, tag="sd")
                nc.scalar.activation(sd, mv[:, 1:2], AF.Sqrt, bias=eps_sb)
                rs = ot.tile([128, 1], F32, tag="rs")
                nc.vector.reciprocal(rs, sd)
                xn = ot.tile([128, D], F32, tag="xn")
                nc.vector.tensor_scalar(
                    out=xn, in0=x, scalar1=mv[:, 0:1], scalar2=rs,
                    op0=mybir.AluOpType.subtract,
                    op1=mybir.AluOpType.mult)
                yt = ot.tile([128, D], F32, tag="yt")
                nc.vector.tensor_mul(yt, xn, ga_sb)
                nc.vector.tensor_add(yt, yt, be_sb)
                nc.sync.dma_start(yout[qb * 128:(qb + 1) * 128, :], yt)


def _get_prog(scales, use_mask, phases="PAO", repeat=1):
    key = (tuple(scales), use_mask, phases, repeat)
    if key not in _prog_cache:
        _prog_cache[key] = _build(scales, use_mask, phases, repeat)
    return _prog_cache[key]


def _prep_inputs(query, key, value, mask, Wq, Wk, Wv, Wo, bo, cmw, gamma,
                 beta):
    f32 = np.float32
    query = np.asarray(query, f32)
    key = np.asarray(key, f32)
    value = np.asarray(value, f32)
    mask = np.asarray(mask)
    cmw = np.asarray(cmw, f32).reshape(H)
    scales = [float(cmw[h]) / math.sqrt(DK) for h in range(H)]
    use_mask = not bool((mask != 0).all())

    wqT = np.ascontiguousarray(np.asarray(Wq, f32).T)
    wkT = np.ascontiguousarray(np.asarray(Wk, f32).T)
    wvT = np.ascontiguousarray(np.asarray(Wv, f32).T)
    woT = np.ascontiguousarray(np.asarray(Wo, f32).T)
    bo1 = np.asarray(bo, f32).reshape(1, D)
    ga1 = np.asarray(gamma, f32).reshape(1, D)
    be1 = np.asarray(beta, f32).reshape(1, D)

    in_maps = []
    for c in range(NCORES):
        b, qh = divmod(c, 2)
        qs = slice(qh * QH, (qh + 1) * QH)
        m = {
            "qT": np.ascontiguousarray(query[b].T[:, qs]),
            "kT": np.ascontiguousarray(key[b].T),
            "vT": np.ascontiguousarray(value[b].T),
            "qnat": np.ascontiguousarray(query[b, qs] + bo1),
            "wqT": wqT, "wkT": wkT, "wvT": wvT, "woT": woT,
            "gamma": ga1, "beta": be1,
        }
        if use_mask:
            m["maskT"] = np.ascontiguousarray(
                mask[b, 0].astype(np.float16).T[:, qs])
        in_maps.append(m)
    return in_maps, scales, use_mask


def run(trace=False, **inputs):
    in_maps, scales, use_mask = _prep_inputs(**inputs)
    nc = _get_prog(scales, use_mask)
    try:
        res = run_bass_kernel_spmd(nc, in_maps, list(range(NCORES)),
                                   trace=trace)
    except ModuleNotFoundError:
        res = run_bass_kernel_spmd(nc, in_maps, list(range(NCORES)),
                                   trace=False)

    y = np.empty((B, SQ, D), np.float32)
    attn = np.empty((B, H, SQ, SK), np.float32)
    for c in range(NCORES):
        b, qh = divmod(c, 2)
        qs = slice(qh * QH, (qh + 1) * QH)
        y[b, qs] = res.results[c]["yout"]
        attn[b, :, qs, :] = res.results[c]["attnT"].transpose(0, 2, 1)
    return (y, attn), res


def kernel(**inputs):
    out, _ = run(trace=False, **inputs)
    return out


def bench(iters=5, phases="PAO", repeat=1, **inputs):
    """Repeat-execute the NEFF on all 8 cores with device-resident inputs;
    returns (list of per-iteration seconds, outputs-of-last-iter results)."""
    import time

    import jax
    from jax.experimental.shard_map import shard_map
    from jax.sharding import Mesh, NamedSharding, PartitionSpec

    from concourse import bass2jax, mybir as mb
    from concourse.bass2jax import _bass_exec_p, install_neuronx_cc_hook

    install_neuronx_cc_hook()
    in_maps, scales, use_mask = _prep_inputs(**inputs)
    nc = _get_prog(scales, use_mask, phases, repeat)

    partition_name = (nc.partition_id_tensor.name
                      if nc.partition_id_tensor else None)
    in_names, out_names, out_avals, zero_outs = [], [], [], []
    for alloc in nc.m.functions[0].allocations:
        if not isinstance(alloc, mb.MemoryLocationSet):
            continue
        name = alloc.memorylocations[0].name
        if alloc.kind == "ExternalInput":
            if name != partition_name:
                in_names.append(name)
        elif alloc.kind == "ExternalOutput":
            out_names.append(name)
            shape = tuple(alloc.tensor_shape)
            dtype = mb.dt.np(alloc.dtype)
            out_avals.append(jax.core.ShapedArray(shape, dtype))
            zero_outs.append(np.zeros(shape, dtype))
    n_params = len(in_names)
    n_outs = len(out_avals)
    in_names_all = list(in_names) + out_names
    if partition_name is not None:
        in_names_all.append(partition_name)

    def _body(*args):
        operands = list(args)
        if partition_name is not None:
            operands.append(bass2jax.partition_id_tensor())
        outs = _bass_exec_p.bind(
            *operands, out_avals=tuple(out_avals),
            in_names=tuple(in_names_all), out_names=tuple(out_names),
            lowering_input_output_aliases=(), sim_require_finite=True,
            sim_require_nnan=True, nc=nc)
        return tuple(outs)

    devices = jax.devices()[:NCORES]
    mesh = Mesh(np.asarray(devices), ("core",))
    in_specs = (PartitionSpec("core"),) * (n_params + n_outs)
    out_specs = (PartitionSpec("core"),) * n_outs
    fn = jax.jit(shard_map(_body, mesh=mesh, in_specs=in_specs,
                           out_specs=out_specs, check_rep=False),
                 keep_unused=True)
    sh = NamedSharding(mesh, PartitionSpec("core"))
    args = []
    for i in range(n_params):
        args.append(jax.device_put(np.concatenate(
            [np.asarray(in_maps[c][in_names[i]]) for c in range(NCORES)],
            axis=0), sh))
    for z in zero_outs:
        args.append(jax.device_put(
            np.zeros((NCORES * z.shape[0], *z.shape[1:]), z.dtype), sh))
    times = []
    out = None
    for _ in range(iters):
        t0 = time.time()
        out = fn(*args)
        jax.block_until_ready(out)
        times.append(time.time() - t0)
    return times, out


# revision 29
# speedup vs baseline: 1.7680x; 1.7680x over previous
"""CrossModalAttention Trainium2 kernel.

Math (per batch b):
    Q = query @ Wq.T ; K = key @ Wk.T ; V = value @ Wv.T        (per-head dk=64)
    scores = (Q K^T) / sqrt(dk) * cmw[h]   (+ mask)
    attn   = softmax(scores, axis=k)
    ctx    = attn @ V
    out    = ctx @ Wo.T + bo
    y      = LayerNorm(out + query) * gamma + beta
Returns (y, attn).

Sharding: 8 cores = 4 batches x 2 query-halves (512 q rows each). No
collectives: every core computes its q-rows end-to-end (K/V projections are
duplicated across the 2 cores that share a batch).

On-device layout is "transposed" ([feature, token]) so every matmul contracts
over the partition dim with no on-chip transposes:
  - QT[f,q], KT[f,k]  from host-transposed activations and W.T
  - scoresT[k,q] = K_h @ Q_h^T   (lhsT = KT chunk, rhs = QT chunk); the two
    heads of a 128-partition pair use disjoint PE row groups and interleave
  - expT = exp(scoresT * scale)        (ACT, fp16)
  - [ctxT; sums] = [V_h | 1]^T @ expT  (ones column fuses the softmax row-sums)
  - recip broadcast over partitions via a rank-1 matmul (ones outer product)
  - attnT = expT * recip  -> DRAM fp16 (host transposes to [q,k] on gather)
  - ctxT normalized, out = ctx @ Wo.T via lhsT = ctxT chunks
  - residual + LN in natural [q, f] layout (bo pre-folded into the residual
    input on the host).
"""

import math

import numpy as np

import concourse.bass as bass
import concourse.mybir as mybir
import concourse.tile as tile
from concourse import bacc
from concourse.bass_utils import run_bass_kernel_spmd

F32 = mybir.dt.float32
F32R = mybir.dt.float32r
F16 = mybir.dt.float16
AF = mybir.ActivationFunctionType

D = 1024          # d_model
H = 16            # heads
DK = 64           # head dim
B = 4
SQ = 1024
SK = 1024
NCORES = 8
QH = SQ // 2      # q rows per core
LN_EPS = 1e-5

_prog_cache = {}


def _build(scales, use_mask, phases="PAO", repeat=1, ln_identity=False):
    nc = bacc.Bacc("TRN2", target_bir_lowering=False, debug=False,
                   num_devices=NCORES)

    qT = nc.dram_tensor("qT", [D, QH], F32R, kind="ExternalInput").ap()
    kT = nc.dram_tensor("kT", [D, SK], F32R, kind="ExternalInput").ap()
    vT = nc.dram_tensor("vT", [D, SK], F32R, kind="ExternalInput").ap()
    qnat = nc.dram_tensor("qnat", [QH, D], F32, kind="ExternalInput").ap()
    wqT = nc.dram_tensor("wqT", [D, D], F32R, kind="ExternalInput").ap()
    wkT = nc.dram_tensor("wkT", [D, D], F32R, kind="ExternalInput").ap()
    wvT = nc.dram_tensor("wvT", [D, D], F32R, kind="ExternalInput").ap()
    woT = nc.dram_tensor("woT", [D, D], F32R, kind="ExternalInput").ap()
    gamma = beta = None
    if not ln_identity:
        gamma = nc.dram_tensor("gamma", [1, D], F32,
                               kind="ExternalInput").ap()
        beta = nc.dram_tensor("beta", [1, D], F32,
                              kind="ExternalInput").ap()
    maskT = None
    if use_mask:
        maskT = nc.dram_tensor("maskT", [SK, QH], F16,
                               kind="ExternalInput").ap()

    attnT = nc.dram_tensor("attnT", [H, SK, QH], F16,
                           kind="ExternalOutput").ap()
    yout = nc.dram_tensor("yout", [QH, D], F32, kind="ExternalOutput").ap()

    def chunked(ap, inner):
        # [ (c p), n ] -> [ p, c, n ] with p=128 partitions
        return ap.rearrange("(c p) n -> p c n", p=128)

    with tile.TileContext(nc) as tc:
      for _rep in range(repeat):
        with tc.tile_pool(name="persist", bufs=1) as pers:
            QT_l = [pers.tile([128, QH], F32R, tag=f"QT{i}", name=f"QT{i}") for i in range(8)]
            KT_l = [pers.tile([128, SK], F32R, tag=f"KT{i}", name=f"KT{i}") for i in range(8)]
            Vo_l = [pers.tile([128, H * 65], F16, tag=f"Vo{i}",
                              name=f"Vo{i}") for i in range(8)]
            ones_sb = pers.tile([1, 128], F32, tag="ones")
            nc.vector.memset(ones_sb, 1.0)
            # ones columns of the [V_h | 1] blocks; V copies overwrite the rest
            for i in range(8):
                nc.vector.memset(Vo_l[i], 1.0)

            # ---------------- Phase P: projections ----------------
            with tc.tile_pool(name="acts", bufs=1) as actp, \
                 tc.tile_pool(name="wts", bufs=1) as wp, \
                 tc.tile_pool(name="vstream", bufs=3) as vsp, \
                 tc.tile_pool(name="pp", bufs=3, space="PSUM") as pp:

                # per-d-chunk DMAs so d=0 matmuls start before the whole
                # tensor has landed
                qT_sb = actp.tile([128, 8, QH], F32R, tag="qT")
                for d in range(8):
                    nc.sync.dma_start(qT_sb[:, d, :], chunked(qT, QH)[:, d, :])
                kT_sb = actp.tile([128, 8, SK], F32R, tag="kT")
                for d in range(8):
                    nc.sync.dma_start(kT_sb[:, d, :], chunked(kT, SK)[:, d, :])

                # QT[f,q] = Wq @ query^T
                wq_sb = wp.tile([128, 8, D], F32R, tag="wA")
                for d in range(8):
                    nc.gpsimd.dma_start(wq_sb[:, d, :], chunked(wqT, D)[:, d, :])
                for fb in range(8):
                    ps = pp.tile([128, SK], F32, tag="pj")
                    for d in range(8):
                        nc.tensor.matmul(
                            ps[:, :QH],
                            lhsT=wq_sb[:, d, fb * 128:(fb + 1) * 128],
                            rhs=qT_sb[:, d, :],
                            start=(d == 0), stop=(d == 7))
                    nc.vector.tensor_copy(QT_l[fb][:], ps[:, :QH])

                # KT[f,k] = Wk @ key^T
                wk_sb = wp.tile([128, 8, D], F32R, tag="wB")
                for d in range(8):
                    nc.gpsimd.dma_start(wk_sb[:, d, :], chunked(wkT, D)[:, d, :])
                for fb in range(8):
                    ps = pp.tile([128, SK], F32, tag="pj")
                    for hf in range(2):
                        for d in range(8):
                            nc.tensor.matmul(
                                ps[:, hf * 512:(hf + 1) * 512],
                                lhsT=wk_sb[:, d, fb * 128:(fb + 1) * 128],
                                rhs=kT_sb[:, d, hf * 512:(hf + 1) * 512],
                                start=(d == 0), stop=(d == 7))
                    nc.vector.tensor_copy(KT_l[fb][:], ps)

                # V[k,f] = value @ Wv.T, written into 65-col [V_h | 1] slots
                wv_sb = wp.tile([128, 8, D], F32R, tag="wA")
                for d in range(8):
                    nc.gpsimd.dma_start(wv_sb[:, d, :], chunked(wvT, D)[:, d, :])
                for kb in range(8):
                    ps = pp.tile([128, SK], F32, tag="pj")
                    vchunk = vsp.tile([128, 8, 128], F32R, tag="vs")
                    nc.sync.dma_start(
                        vchunk, chunked(vT, SK)[:, :, kb * 128:(kb + 1) * 128])
                    for hf in range(2):
                        for d in range(8):
                            nc.tensor.matmul(
                                ps[:, hf * 512:(hf + 1) * 512],
                                lhsT=vchunk[:, d, :],
                                rhs=wv_sb[:, d, hf * 512:(hf + 1) * 512],
                                start=(d == 0), stop=(d == 7))
                    src = ps.rearrange("p (h c) -> p h c", c=DK)
                    dst = Vo_l[kb][:].rearrange("p (h c) -> p h c", c=65)
                    nc.vector.tensor_copy(dst[:, :, 0:DK], src)

            # ---------------- Phases A + O ----------------
            if "A" in phases:
                _phases_ao(nc, tc, QT_l, KT_l, Vo_l, ones_sb,
                           scales, use_mask, maskT, qnat, woT, gamma,
                           beta, attnT, yout, phases, chunked)

    nc.compile()
    return nc


def _head_tail(nc, h, expP, hp, Vo_l, ctxT_sb, attnT, an, pb):
    """ctx+sums matmul, softmax normalization, attnT store for one head."""
    h2 = h // 2
    # [ctxT ; sums] = [V_h | 1]^T @ expT
    ps_ct = pb.tile([128, QH], F32, tag="ct")
    for kb in range(8):
        nc.tensor.matmul(
            ps_ct[0:65, :],
            lhsT=Vo_l[kb][:, h * 65:(h + 1) * 65],
            rhs=expP[:, kb, hp, :],
            start=(kb == 0), stop=(kb == 7))

    # sums live at psum partition 64; recip_approx_fast breaks at
    # partition base != 0 on HW, so hop the row down to partition 0
    # (ACT copy -> tiny DMA) first.
    sums_hi = an.tile([65, QH], F32, tag="sh")
    nc.scalar.copy(sums_hi[64:65, :], ps_ct[64:65, :])
    sums_lo = an.tile([1, QH], F32, tag="sl")
    nc.sync.dma_start(sums_lo[:], sums_hi[64:65, :])
    rec = an.tile([1, QH], F32, tag="rec")
    nc.vector.reciprocal_approx_fast(rec[:], sums_lo[:])
    rec16 = an.tile([1, QH], F16, tag="rec16")
    nc.vector.tensor_copy(rec16[:], rec[:])
    b32 = an.tile([64, QH], F32, tag="b32")
    nc.gpsimd.partition_broadcast(b32[:], rec[:])
    b16 = an.tile([128, QH], F16, tag="b16")
    nc.gpsimd.partition_broadcast(b16[:], rec16[:])

    # normalized ctxT into the head-pair partition slot
    if hp == 0:
        nc.vector.tensor_mul(ctxT_sb[0:64, h2, :], ps_ct[0:64, :],
                             b32[0:64, :])
    else:
        stg = an.tile([64, QH], F32R, tag="stg")
        nc.vector.tensor_mul(stg, ps_ct[0:64, :], b32[0:64, :])
        nc.sync.dma_start(ctxT_sb[64:128, h2, :], stg)

    # attnT out (one batched DMA per head)
    at = an.tile([128, 8, QH], F16, tag="at")
    for kb in range(8):
        nc.vector.tensor_mul(at[:, kb, :], expP[:, kb, hp, :], b16)
    nc.sync.dma_start(attnT[h].rearrange("(c p) q -> p c q", p=128), at)


def _phases_ao(nc, tc, QT_l, KT_l, Vo_l, ones_sb, scales, use_mask,
               maskT, qnat, woT, gamma, beta, attnT, yout, phases, chunked):
    with tc.tile_pool(name="late", bufs=1) as late:
        ctxT_sb = late.tile([128, 8, QH], F32R, tag="ctxT")
        wo_sb = late.tile([128, 8, D], F32R, tag="wo")
        for d in range(8):
            nc.gpsimd.dma_start(wo_sb[:, d, :], chunked(woT, D)[:, d, :])
        qn_sb = late.tile([128, 4, D], F32, tag="qn")
        for c in range(4):
            nc.sync.dma_start(qn_sb[:, c, :], chunked(qnat, D)[:, c, :])
        with tc.tile_pool(name="ah", bufs=(1 if use_mask else 2)) as ah, \
             tc.tile_pool(name="an", bufs=2) as an, \
             tc.tile_pool(name="pa", bufs=3, space="PSUM") as pa, \
             tc.tile_pool(name="pb", bufs=2, space="PSUM") as pb, \
             (tc.tile_pool(name="mp", bufs=1) if use_mask else
              tc.tile_pool(name="mp0", bufs=1)) as mp:

            mask_sb = None
            if use_mask:
                mask_sb = mp.tile([128, 8, QH], F16, tag="mk")
                nc.sync.dma_start(mask_sb, chunked(maskT, QH))

            for h2 in range(8):
                # the two heads of the pair use PE row groups 0-63 / 64-127,
                # so their interleaved scoresT matmuls run concurrently; both
                # land in one 2-bank psum tile so one ACT instruction
                # exponentiates the pair (scales are equal for all heads in
                # the fast path enforced by _prep_inputs).
                expP = ah.tile([128, 8, 2, QH], F16, tag="expP")
                for kb in range(8):
                    ps_s = pa.tile([128, 2, QH], F32, tag="s")
                    for hp in (0, 1):
                        base = hp * 64
                        nc.tensor.matmul(
                            ps_s[:, hp, :],
                            lhsT=KT_l[h2][base:base + 64,
                                          kb * 128:(kb + 1) * 128],
                            rhs=QT_l[h2][base:base + 64, :],
                            start=True, stop=True)
                    if scales[2 * h2] == scales[2 * h2 + 1]:
                        nc.scalar.activation(expP[:, kb, :, :], ps_s,
                                             AF.Exp, scale=scales[2 * h2])
                    else:
                        for hp in (0, 1):
                            nc.scalar.activation(
                                expP[:, kb, hp, :], ps_s[:, hp, :],
                                AF.Exp, scale=scales[2 * h2 + hp])
                    if use_mask:
                        for hp in (0, 1):
                            nc.vector.tensor_mul(expP[:, kb, hp, :],
                                                 expP[:, kb, hp, :],
                                                 mask_sb[:, kb, :])

                for hp in (0, 1):
                    _head_tail(nc, 2 * h2 + hp, expP, hp, Vo_l,
                               ctxT_sb, attnT, an, pb)

        # ---------------- Phase O ----------------
        if "O" not in phases:
            return
        with tc.tile_pool(name="oc", bufs=1) as oc, \
             tc.tile_pool(name="ot", bufs=2) as ot, \
             tc.tile_pool(name="po", bufs=2, space="PSUM") as po:

            ga_sb = be_sb = None
            if gamma is not None:
                ga_sb = oc.tile([128, D], F32, tag="ga")
                nc.sync.dma_start(ga_sb, gamma.to_broadcast([128, D]))
                be_sb = oc.tile([128, D], F32, tag="be")
                nc.sync.dma_start(be_sb, beta.to_broadcast([128, D]))
            eps_sb = oc.tile([128, 1], F32, tag="eps")
            nc.vector.memset(eps_sb, LN_EPS)

            for qb in range(4):
                ps_o = po.tile([128, D], F32, tag="out")
                for hf in range(2):
                    for d in range(8):
                        nc.tensor.matmul(
                            ps_o[:, hf * 512:(hf + 1) * 512],
                            lhsT=ctxT_sb[:, d, qb * 128:(qb + 1) * 128],
                            rhs=wo_sb[:, d, hf * 512:(hf + 1) * 512],
                            start=(d == 0), stop=(d == 7))
                x = ot.tile([128, D], F32, tag="x")
                # qn_sb already holds query-rows + bo (host-folded)
                nc.vector.tensor_add(x, ps_o, qn_sb[:, qb, :])

                stats = ot.tile([128, 2, 6], F32, tag="st")
                for sg in range(2):
                    nc.vector.bn_stats(stats[:, sg, :],
                                       x[:, sg * 512:(sg + 1) * 512])
                mv = ot.tile([128, 2], F32, tag="mv")
                nc.vector.bn_aggr(mv, stats)
                sd = ot.tile([128, 1], F32, tag="sd")
                nc.scalar.activation(sd, mv[:, 1:2], AF.Sqrt, bias=eps_sb)
                rs = ot.tile([128, 1], F32, tag="rs")
                nc.vector.reciprocal(rs, sd)
                xn = ot.tile([128, D], F32, tag="xn")
                nc.vector.tensor_scalar(
                    out=xn, in0=x, scalar1=mv[:, 0:1], scalar2=rs,
                    op0=mybir.AluOpType.subtract,
                    op1=mybir.AluOpType.mult)
                if ga_sb is not None:
                    yt = ot.tile([128, D], F32, tag="yt")
                    nc.vector.tensor_mul(yt, xn, ga_sb)
                    nc.vector.tensor_add(yt, yt, be_sb)
                else:
                    yt = xn
                nc.sync.dma_start(yout[qb * 128:(qb + 1) * 128, :], yt)


def _get_prog(scales, use_mask, phases="PAO", repeat=1, ln_identity=False):
    key = (tuple(scales), use_mask, phases, repeat, ln_identity)
    if key not in _prog_cache:
        _prog_cache[key] = _build(scales, use_mask, phases, repeat,
                                  ln_identity)
    return _prog_cache[key]


def _prep_inputs(query, key, value, mask, Wq, Wk, Wv, Wo, bo, cmw, gamma,
                 beta):
    f32 = np.float32
    query = np.asarray(query, f32)
    key = np.asarray(key, f32)
    value = np.asarray(value, f32)
    mask = np.asarray(mask)
    cmw = np.asarray(cmw, f32).reshape(H)
    scales = [float(cmw[h]) / math.sqrt(DK) for h in range(H)]
    use_mask = not bool((mask != 0).all())

    wqT = np.ascontiguousarray(np.asarray(Wq, f32).T)
    wkT = np.ascontiguousarray(np.asarray(Wk, f32).T)
    wvT = np.ascontiguousarray(np.asarray(Wv, f32).T)
    woT = np.ascontiguousarray(np.asarray(Wo, f32).T)
    bo1 = np.asarray(bo, f32).reshape(1, D)
    ga1 = np.asarray(gamma, f32).reshape(1, D)
    be1 = np.asarray(beta, f32).reshape(1, D)
    ln_identity = bool((ga1 == 1.0).all() and (be1 == 0.0).all())

    in_maps = []
    for c in range(NCORES):
        b, qh = divmod(c, 2)
        qs = slice(qh * QH, (qh + 1) * QH)
        m = {
            "qT": np.ascontiguousarray(query[b].T[:, qs]),
            "kT": np.ascontiguousarray(key[b].T),
            "vT": np.ascontiguousarray(value[b].T),
            "qnat": np.ascontiguousarray(query[b, qs] + bo1),
            "wqT": wqT, "wkT": wkT, "wvT": wvT, "woT": woT,
        }
        if not ln_identity:
            m["gamma"] = ga1
            m["beta"] = be1
        if use_mask:
            m["maskT"] = np.ascontiguousarray(
                mask[b, 0].astype(np.float16).T[:, qs])
        in_maps.append(m)
    return in_maps, scales, use_mask, ln_identity


def run(trace=False, **inputs):
    in_maps, scales, use_mask, ln_id = _prep_inputs(**inputs)
    nc = _get_prog(scales, use_mask, ln_identity=ln_id)
    try:
        res = run_bass_kernel_spmd(nc, in_maps, list(range(NCORES)),
                                   trace=trace)
    except ModuleNotFoundError:
        res = run_bass_kernel_spmd(nc, in_maps, list(range(NCORES)),
                                   trace=False)

    y = np.empty((B, SQ, D), np.float32)
    attn = np.empty((B, H, SQ, SK), np.float32)
    for c in range(NCORES):
        b, qh = divmod(c, 2)
        qs = slice(qh * QH, (qh + 1) * QH)
        y[b, qs] = res.results[c]["yout"]
        attn[b, :, qs, :] = res.results[c]["attnT"].transpose(0, 2, 1)
    return (y, attn), res


def kernel(**inputs):
    out, _ = run(trace=False, **inputs)
    return out


def bench(iters=5, phases="PAO", repeat=1, **inputs):
    """Repeat-execute the NEFF on all 8 cores with device-resident inputs;
    returns (list of per-iteration seconds, outputs-of-last-iter results)."""
    import time

    import jax
    from jax.experimental.shard_map import shard_map
    from jax.sharding import Mesh, NamedSharding, PartitionSpec

    from concourse import bass2jax, mybir as mb
    from concourse.bass2jax import _bass_exec_p, install_neuronx_cc_hook

    install_neuronx_cc_hook()
    in_maps, scales, use_mask, ln_id = _prep_inputs(**inputs)
    nc = _get_prog(scales, use_mask, phases, repeat, ln_id)

    partition_name = (nc.partition_id_tensor.name
                      if nc.partition_id_tensor else None)
    in_names, out_names, out_avals, zero_outs = [], [], [], []
    for alloc in nc.m.functions[0].allocations:
        if not isinstance(alloc, mb.MemoryLocationSet):
            continue
        name = alloc.memorylocations[0].name
        if alloc.kind == "ExternalInput":
            if name != partition_name:
                in_names.append(name)
        elif alloc.kind == "ExternalOutput":
            out_names.append(name)
            shape = tuple(alloc.tensor_shape)
            dtype = mb.dt.np(alloc.dtype)
            out_avals.append(jax.core.ShapedArray(shape, dtype))
            zero_outs.append(np.zeros(shape, dtype))
    n_params = len(in_names)
    n_outs = len(out_avals)
    in_names_all = list(in_names) + out_names
    if partition_name is not None:
        in_names_all.append(partition_name)

    def _body(*args):
        operands = list(args)
        if partition_name is not None:
            operands.append(bass2jax.partition_id_tensor())
        outs = _bass_exec_p.bind(
            *operands, out_avals=tuple(out_avals),
            in_names=tuple(in_names_all), out_names=tuple(out_names),
            lowering_input_output_aliases=(), sim_require_finite=True,
            sim_require_nnan=True, nc=nc)
        return tuple(outs)

    devices = jax.devices()[:NCORES]
    mesh = Mesh(np.asarray(devices), ("core",))
    in_specs = (PartitionSpec("core"),) * (n_params + n_outs)
    out_specs = (PartitionSpec("core"),) * n_outs
    fn = jax.jit(shard_map(_body, mesh=mesh, in_specs=in_specs,
                           out_specs=out_specs, check_rep=False),
                 keep_unused=True)
    sh = NamedSharding(mesh, PartitionSpec("core"))
    args = []
    for i in range(n_params):
        args.append(jax.device_put(np.concatenate(
            [np.asarray(in_maps[c][in_names[i]]) for c in range(NCORES)],
            axis=0), sh))
    for z in zero_outs:
        args.append(jax.device_put(
            np.zeros((NCORES * z.shape[0], *z.shape[1:]), z.dtype), sh))
    times = []
    out = None
    for _ in range(iters):
        t0 = time.time()
        out = fn(*args)
        jax.block_until_ready(out)
        times.append(time.time() - t0)
    return times, out
